# revision 1
# baseline (speedup 1.0000x reference)
"""Trainium2 Bass kernel for nn_Model2_3925600109170 (gnn_message_passing).

Only the news->news GAT + MLP head + final row-gather affect the output
(the SAGE and news->topic GAT results are computed then deleted in the
reference), so this kernel implements:

    hs = x_news @ ws.T ; es = hs @ a_s ; ed = (x_news @ wd.T) @ a_d
    e  = leaky_relu(es[src] + ed[dst], 0.2)      (softmax max-shift skipped:
    w  = exp(e)                                   |e| <= ~3, exp safe in f32,
    num= segsum(w * hs[src]); den = segsum(w)     ratio is shift-invariant)
    h  = num / max(den, 1e-16) + b
    out= relu(h @ W1.T + b1) @ W2.T + b2 ; return out[news_indices]

Sharding: dst-range partitioning over 8 cores (12500 dst rows each).
Each core builds a replicated bf16 row table [hs(64) | 1 | es | pad] with
256-byte rows (dma_gather elem), gathers per-edge rows + per-edge ed, and
does the segment softmax-sum as one-hot matmuls accumulated in PSUM per
128-dst block, then the MLP fused per block.
"""

import numpy as np

N_NEWS = 100_000
D = 128
H = 64
N_PER_CORE = 12_500           # dst rows per core
N_BLK = 98                    # ceil(12500/128)
BLK_PER_SC = 7                # blocks per superchunk
N_SC = 14                     # 98 = 14 * 7
QW = 25_088                   # src window width (196*128), 4 windows
N_Q = 4
XT_COLS = QW * N_Q            # 100352 padded x columns
ED_ROWS = N_BLK * 128         # 12544

_CACHE = {}


def _host_prep(x_news, ws, a_s, wd, a_d, b, w1, b1, w2, b2,
               links_src, links_dst):
    """Build per-core input maps + the shared compile-time schedule."""
    f32 = np.float32

    xt = np.zeros((D, XT_COLS), f32)
    xt[:, :N_NEWS] = np.ascontiguousarray(x_news.T)

    wprime = np.zeros((D, 66), f32)
    wprime[:, 0:64] = ws.T
    wprime[:, 65] = ws.T @ a_s
    wdprime = (wd.T @ a_d).astype(f32).reshape(D, 1)

    w1t = np.ascontiguousarray(w1.T).astype(f32)          # [64, 64]
    b1p = (w1 @ b + b1).astype(f32).reshape(H, 1)
    w2t = np.ascontiguousarray(w2.T).astype(f32)          # [64, 32]
    b2c = b2.astype(f32).reshape(32, 1)
    iota = np.broadcast_to(np.arange(128, dtype=f32), (128, 128)).copy()

    src = links_src.astype(np.int64)
    dst = links_dst.astype(np.int64)
    core_of = dst // N_PER_CORE
    dst_local = dst - core_of * N_PER_CORE
    blk = dst_local >> 7
    dib = dst_local & 127                     # dst index within block
    qtr = src // QW
    sc = blk // BLK_PER_SC
    bis = blk % BLK_PER_SC                    # block within superchunk
    # group key per edge (within its core): schedule order (sc, qtr, bis)
    gkey = (sc * N_Q + qtr) * BLK_PER_SC + bis
    NG = N_SC * N_Q * BLK_PER_SC              # 392 groups

    counts = np.zeros((8, NG), np.int64)
    for c in range(8):
        counts[c] = np.bincount(gkey[core_of == c], minlength=NG)
    kchunks = (np.maximum(counts.max(axis=0), (counts.max(axis=0) == 0)) + 127) // 128
    # ensure every block has >= 1 chunk overall (degenerate safety)
    # kchunks[g] >= 1 already forced above where all-zero.

    chunk_off = np.zeros(NG + 1, np.int64)
    np.cumsum(kchunks, out=chunk_off[1:])
    C_TOT = int(chunk_off[-1])
    SLOTS = C_TOT * 128

    # schedule metadata for codegen
    sched = []      # per sc: dict(q_chunks=[4], chunks=[(block_in_sc)], c0=chunk offset)
    for s in range(N_SC):
        q_chunks = []
        chunk_blocks = []
        for q in range(N_Q):
            nq = 0
            for bi in range(BLK_PER_SC):
                g = (s * N_Q + q) * BLK_PER_SC + bi
                k = int(kchunks[g])
                nq += k
                chunk_blocks += [bi] * k
            q_chunks.append(nq)
        sched.append(dict(
            c0=int(chunk_off[(s * N_Q) * BLK_PER_SC]),
            q_chunks=q_chunks,
            chunk_blocks=chunk_blocks,
        ))

    def idx_tile(arr_i16):
        # [n] -> [128, n/16] wrapped 16-partition layout replicated 8x
        n = arr_i16.shape[0]
        t = arr_i16.reshape(n // 16, 16).T      # [16, n/16]
        return np.tile(t, (8, 1))               # [128, n/16]

    in_maps = []
    for c in range(8):
        m = core_of == c
        e_src = src[m]
        e_g = gkey[m]
        e_dib = dib[m]
        e_dl = dst_local[m]
        e_q = qtr[m]
        order = np.argsort(e_g, kind="stable")
        e_src, e_g, e_dib, e_dl, e_q = (a[order] for a in (e_src, e_g, e_dib, e_dl, e_q))
        gstart = np.zeros(NG + 1, np.int64)
        np.cumsum(np.bincount(e_g, minlength=NG), out=gstart[1:])
        rank = np.arange(e_g.shape[0]) - gstart[e_g]
        slot = chunk_off[e_g] * 128 + rank

        hs_local = np.zeros(SLOTS, np.int16)          # pad: row 0 of window
        ed_local = np.zeros(SLOTS, np.int16)          # pad: row 0
        dstl = np.full(SLOTS, -1.0, f32)              # pad: no dst match
        hs_local[slot] = (e_src - e_q * QW).astype(np.int16)
        ed_local[slot] = e_dl.astype(np.int16)
        dstl[slot] = e_dib.astype(f32)

        # hs idx tiles: per (sc, quarter) gather; ed idx: per sc gather.
        hs_cols, ed_cols = [], []
        for s in range(N_SC):
            base = sched[s]["c0"] * 128
            nsc = sum(sched[s]["q_chunks"]) * 128
            off = base
            for q in range(N_Q):
                nq = sched[s]["q_chunks"][q] * 128
                if nq:
                    hs_cols.append(idx_tile(hs_local[off:off + nq]))
                off += nq
            ed_cols.append(idx_tile(ed_local[base:base + nsc]))
        hsidx = np.concatenate(hs_cols, axis=1)
        edidx = np.concatenate(ed_cols, axis=1)
        dstlf = np.ascontiguousarray(dstl.reshape(C_TOT, 128).T)  # [128, C_TOT]

        in_maps.append(dict(
            xt=xt, xto=np.ascontiguousarray(xt[:, c * N_PER_CORE: c * N_PER_CORE + ED_ROWS]),
            wprime=wprime, wdprime=wdprime, w1t=w1t, b1p=b1p, w2t=w2t, b2c=b2c,
            iota=iota.astype(np.float32),
            hsidx=hsidx, edidx=edidx, dstlf=dstlf,
        ))

    shapes = dict(S_HS=in_maps[0]["hsidx"].shape[1], S_ED=in_maps[0]["edidx"].shape[1],
                  C_TOT=C_TOT)
    return in_maps, sched, shapes


def _build_program(sched, shapes, n_sc_run=N_SC, n_repeat=1, p2_mode='full'):
    import concourse.bass as bass
    import concourse.bacc as bacc
    import concourse.mybir as mybir
    import concourse.tile as tile

    f32, bf16, i16 = mybir.dt.float32, mybir.dt.bfloat16, mybir.dt.int16
    AO = mybir.AluOpType
    AF = mybir.ActivationFunctionType

    nc = bacc.Bacc("TRN2", target_bir_lowering=False, debug=False, num_devices=8)

    xt = nc.dram_tensor("xt", [D, XT_COLS], f32, kind="ExternalInput")
    xto = nc.dram_tensor("xto", [D, ED_ROWS], f32, kind="ExternalInput")
    wprime = nc.dram_tensor("wprime", [D, 66], f32, kind="ExternalInput")
    wdprime = nc.dram_tensor("wdprime", [D, 1], f32, kind="ExternalInput")
    w1t = nc.dram_tensor("w1t", [H, H], f32, kind="ExternalInput")
    b1p = nc.dram_tensor("b1p", [H, 1], f32, kind="ExternalInput")
    w2t = nc.dram_tensor("w2t", [H, 32], f32, kind="ExternalInput")
    b2c = nc.dram_tensor("b2c", [32, 1], f32, kind="ExternalInput")
    iota = nc.dram_tensor("iota", [128, 128], f32, kind="ExternalInput")
    hsidx = nc.dram_tensor("hsidx", [128, shapes["S_HS"]], i16, kind="ExternalInput")
    edidx = nc.dram_tensor("edidx", [128, shapes["S_ED"]], i16, kind="ExternalInput")
    dstlf = nc.dram_tensor("dstlf", [128, shapes["C_TOT"]], f32, kind="ExternalInput")
    outt = nc.dram_tensor("outt", [32, ED_ROWS], f32, kind="ExternalOutput")

    tabs = [nc.dram_tensor(f"tab{q}", [QW, 128], bf16, kind="Internal")
            for q in range(N_Q)]
    edtab = nc.dram_tensor("edtab", [ED_ROWS, 128], bf16, kind="Internal")

    with tile.TileContext(nc) as tc:
        with tc.tile_pool(name="const", bufs=1) as constp:
            wp_t = constp.tile([D, 66], f32)
            nc.sync.dma_start(out=wp_t[:], in_=wprime.ap())
            wdp_t = constp.tile([D, 1], f32)
            nc.sync.dma_start(out=wdp_t[:], in_=wdprime.ap())
            w1t_t = constp.tile([H, H], f32)
            nc.sync.dma_start(out=w1t_t[:], in_=w1t.ap())
            b1p_t = constp.tile([H, 1], f32)
            nc.sync.dma_start(out=b1p_t[:], in_=b1p.ap())
            w2t_t = constp.tile([H, 32], f32)
            nc.sync.dma_start(out=w2t_t[:], in_=w2t.ap())
            b2c_t = constp.tile([32, 1], f32)
            nc.sync.dma_start(out=b2c_t[:], in_=b2c.ap())
            iota_t = constp.tile([128, 128], f32)
            nc.sync.dma_start(out=iota_t[:], in_=iota.ap())
            ones_t = constp.tile([1, H], f32)
            nc.vector.memset(ones_t[:], 1.0)

            def emit_body():
                hs_off_holder = [0]
                # ---------------- Phase 1: build tables ----------------
                GT = 4  # x tiles per group
                with (
                    tc.tile_pool(name="p1", bufs=3) as p1,
                    tc.tile_pool(name="p1ps", bufs=2, space="PSUM") as p1ps,
                ):
                    n_groups = XT_COLS // (128 * GT)          # 196
                    for g in range(n_groups):
                        xg = p1.tile([D, GT * 128], f32)
                        nc.sync.dma_start(out=xg[:], in_=xt.ap()[:, g * GT * 128:(g + 1) * GT * 128])
                        ps = p1ps.tile([128, GT, 512], f32, space="PSUM")
                        for m in range(GT):
                            nc.tensor.matmul(out=ps[:, m, 0:66],
                                             lhsT=xg[:, m * 128:(m + 1) * 128],
                                             rhs=wp_t[:], start=True, stop=True)
                        sb = p1.tile([128, GT, 66], bf16)
                        nc.vector.tensor_copy(out=sb[:, :, 0:66], in_=ps[:, :, 0:66])
                        nc.vector.tensor_scalar(out=sb[:, :, 64:65], in0=ps[:, :, 64:65],
                                                scalar1=0.0, scalar2=1.0,
                                                op0=AO.mult, op1=AO.add)
                        q, lt = divmod(g, n_groups // N_Q)
                        nc.sync.dma_start(
                            out=tabs[q].ap()[lt * GT * 128:(lt + 1) * GT * 128, 0:66]
                                .rearrange("(t p) c -> p t c", p=128),
                            in_=sb[:])

                # ed table: 98 tiles in groups of 7 (own pools; p1 psum freed)
                with (
                    tc.tile_pool(name="p1b", bufs=3) as p1b,
                    tc.tile_pool(name="p1bps", bufs=2, space="PSUM") as p1bps,
                ):
                    n_eg = N_BLK // 7                          # 14
                    for g in range(n_eg):
                        xg = p1b.tile([D, 7 * 128], f32)
                        nc.sync.dma_start(out=xg[:], in_=xto.ap()[:, g * 7 * 128:(g + 1) * 7 * 128])
                        ps = p1bps.tile([128, 7, 64], f32, space="PSUM")
                        for m in range(7):
                            nc.tensor.matmul(out=ps[:, m, 0:1],
                                             lhsT=xg[:, m * 128:(m + 1) * 128],
                                             rhs=wdp_t[:], start=True, stop=True)
                        sb = p1b.tile([128, 7, 128], bf16)
                        for m in range(7):
                            nc.vector.tensor_copy(
                                out=sb[:, m, :],
                                in_=ps[:, m, 0:1].to_broadcast([128, 128]))
                        nc.sync.dma_start(
                            out=edtab.ap()[g * 7 * 128:(g + 1) * 7 * 128, :]
                                .rearrange("(t p) c -> p t c", p=128),
                            in_=sb[:])

                # ---------------- Phase 2: edges ----------------
                max_nc = max(sum(s["q_chunks"]) for s in sched)
                max_nq = max(max(s["q_chunks"]) for s in sched)
                with (
                    tc.tile_pool(name="gq", bufs=2) as gqp,
                    tc.tile_pool(name="edg", bufs=2) as edp,
                    tc.tile_pool(name="wrk", bufs=2) as wrk,
                    tc.tile_pool(name="sel", bufs=3) as selp,
                    tc.tile_pool(name="blk", bufs=3) as blkp,
                    tc.tile_pool(name="aggps", bufs=2, space="PSUM") as aggps,
                    tc.tile_pool(name="smps", bufs=4, space="PSUM") as smps,
                ):
                    hs_off = 0   # column offset into hsidx (int16 cols)
                    for s in range(n_sc_run):
                        meta = sched[s]
                        n_c = sum(meta["q_chunks"])
                        c0 = meta["c0"]

                        # load dstl columns for this superchunk
                        dstl_t = wrk.tile([128, max_nc], f32, tag="dstl")
                        nc.sync.dma_start(out=dstl_t[:, 0:n_c], in_=dstlf.ap()[:, c0:c0 + n_c])

                        # ed idx + gather (whole superchunk, schedule order)
                        sed = n_c * 8
                        edi_t = wrk.tile([128, max_nc * 8], i16, tag="edi")
                        nc.sync.dma_start(out=edi_t[:, 0:sed],
                                          in_=edidx.ap()[:, c0 * 8:c0 * 8 + sed])
                        ed_g = edp.tile([128, max_nc, 128], bf16, tag="edg")
                        GCH = 64     # chunks per gather (<= 1008 ring descriptors)
                        for gc0 in range(0, n_c, GCH):
                            gn = min(GCH, n_c - gc0)
                            nc.gpsimd.dma_gather(
                                out_ap=ed_g[:, gc0:gc0 + gn, :], in_ap=edtab.ap(),
                                idxs_ap=edi_t[:, gc0 * 8:(gc0 + gn) * 8],
                                num_idxs=gn * 128,
                                num_idxs_reg=gn * 128, elem_size=128,
                                single_packet=False)

                        # hs idx + per-quarter gathers
                        shs = n_c * 8
                        hsi_t = wrk.tile([128, max_nc * 8], i16, tag="hsi")
                        nc.sync.dma_start(out=hsi_t[:, 0:shs],
                                          in_=hsidx.ap()[:, hs_off:hs_off + shs])
                        g_ts = []
                        qoff = 0
                        for q in range(N_Q):
                            nq = meta["q_chunks"][q]
                            g_t = gqp.tile([128, max_nq, 128], bf16, tag=f"g{q}")
                            for gc0 in range(0, nq, 64):
                                gn = min(64, nq - gc0)
                                nc.gpsimd.dma_gather(
                                    out_ap=g_t[:, gc0:gc0 + gn, :], in_ap=tabs[q].ap(),
                                    idxs_ap=hsi_t[:, (qoff + gc0) * 8:(qoff + gc0 + gn) * 8],
                                    num_idxs=gn * 128, num_idxs_reg=gn * 128,
                                    elem_size=128, single_packet=False)
                            g_ts.append(g_t)
                            qoff += nq
                        hs_off += shs

                        if p2_mode == "gather":
                            cons = wrk.tile([32, 640], f32, tag="cons")
                            for q in range(N_Q):
                                nc.vector.tensor_copy(
                                    out=cons[:, q * 128:(q + 1) * 128],
                                    in_=g_ts[q][0:32, 0, :])
                            nc.vector.tensor_copy(out=cons[:, 512:640],
                                                  in_=ed_g[0:32, 0, :])
                            nc.sync.dma_start(
                                out=outt.ap()[:, s * 640:(s + 1) * 640], in_=cons[:])
                            continue

                        # w = exp(leaky_relu(es + ed))
                        es_t = wrk.tile([128, max_nc], f32, tag="es")
                        coff = 0
                        for q in range(N_Q):
                            nq = meta["q_chunks"][q]
                            if nq:
                                nc.vector.tensor_copy(out=es_t[:, coff:coff + nq],
                                                      in_=g_ts[q][:, 0:nq, 65])
                            coff += nq
                        ed_t = wrk.tile([128, max_nc], f32, tag="ed")
                        nc.vector.tensor_copy(out=ed_t[:, 0:n_c], in_=ed_g[:, 0:n_c, 0])
                        l_t = wrk.tile([128, max_nc], f32, tag="l")
                        nc.vector.tensor_tensor(out=l_t[:, 0:n_c], in0=es_t[:, 0:n_c],
                                                in1=ed_t[:, 0:n_c], op=AO.add)
                        t_t = wrk.tile([128, max_nc], f32, tag="t")
                        nc.vector.tensor_scalar_mul(t_t[:, 0:n_c], l_t[:, 0:n_c], 0.2)
                        nc.vector.tensor_tensor(out=l_t[:, 0:n_c], in0=l_t[:, 0:n_c],
                                                in1=t_t[:, 0:n_c], op=AO.max)
                        w_t = wrk.tile([128, max_nc], f32, tag="w")
                        nc.scalar.activation(w_t[:, 0:n_c], l_t[:, 0:n_c], AF.Exp)

                        if p2_mode == "nosel":
                            nc.sync.dma_start(
                                out=outt.ap()[:, s * 640: s * 640 + 128],
                                in_=w_t[0:32, 0:128])
                            continue

                        # map chunk -> (quarter tile, col within tile)
                        qof = [0] * (N_Q + 1)
                        for q in range(N_Q):
                            qof[q + 1] = qof[q] + meta["q_chunks"][q]

                        def chunk_src(ci):
                            for q in range(N_Q):
                                if ci < qof[q + 1]:
                                    return g_ts[q], ci - qof[q]
                            raise AssertionError

                        blk_chunks = [[] for _ in range(BLK_PER_SC)]
                        for ci, bi in enumerate(meta["chunk_blocks"]):
                            blk_chunks[bi].append(ci)

                        # block-major aggregation + per-block normalize + MLP
                        osb = blkp.tile([32, BLK_PER_SC, 128], f32, tag="osb")
                        for bi in range(BLK_PER_SC):
                            aggp = aggps.tile([66, 128], f32, space="PSUM", tag="agg")
                            for k, ci in enumerate(blk_chunks[bi]):
                                g_t, col = chunk_src(ci)
                                sel = selp.tile([128, 128], bf16, tag="sel")
                                nc.vector.tensor_scalar(
                                    out=sel[:], in0=iota_t[:],
                                    scalar1=dstl_t[:, ci:ci + 1], scalar2=w_t[:, ci:ci + 1],
                                    op0=AO.is_equal, op1=AO.mult)
                                nc.tensor.matmul(
                                    out=aggp[:], lhsT=g_t[:, col, 0:66], rhs=sel[:],
                                    start=(k == 0), stop=(k == len(blk_chunks[bi]) - 1))
                            if p2_mode == "nomlp":
                                nc.vector.tensor_copy(out=osb[:, bi, :], in_=aggp[0:32, :])
                                continue
                            den_t = blkp.tile([1, 128], f32, tag="den")
                            nc.vector.tensor_scalar_max(den_t[:], aggp[64:65, :], 1e-16)
                            rec_t = blkp.tile([1, 128], f32, tag="rec")
                            nc.vector.reciprocal(rec_t[:], den_t[:])
                            rbc_p = smps.tile([H, 128], f32, space="PSUM", tag="sm")
                            nc.tensor.matmul(out=rbc_p[:], lhsT=ones_t[:], rhs=rec_t[:],
                                             start=True, stop=True)
                            rbc_t = blkp.tile([H, 128], f32, tag="rbc")
                            nc.vector.tensor_copy(out=rbc_t[:], in_=rbc_p[:])
                            ht_t = blkp.tile([H, 128], f32, tag="ht")
                            nc.vector.tensor_tensor(out=ht_t[:], in0=aggp[0:64, :],
                                                    in1=rbc_t[:], op=AO.mult)
                            mm1_p = smps.tile([H, 128], f32, space="PSUM", tag="sm")
                            nc.tensor.matmul(out=mm1_p[:], lhsT=w1t_t[:], rhs=ht_t[:],
                                             start=True, stop=True)
                            x1_t = blkp.tile([H, 128], f32, tag="x1")
                            nc.scalar.activation(x1_t[:], mm1_p[:], AF.Relu,
                                                 bias=b1p_t[:], scale=1.0)
                            mm2_p = smps.tile([32, 128], f32, space="PSUM", tag="sm")
                            nc.tensor.matmul(out=mm2_p[:], lhsT=w2t_t[:], rhs=x1_t[:],
                                             start=True, stop=True)
                            nc.vector.tensor_scalar(out=osb[:, bi, :], in0=mm2_p[:],
                                                    scalar1=b2c_t[:], scalar2=None,
                                                    op0=AO.add)
                        nc.sync.dma_start(
                            out=outt.ap()[:, s * BLK_PER_SC * 128:(s + 1) * BLK_PER_SC * 128],
                            in_=osb[:])


            for _rep in range(n_repeat):
                emit_body()

    nc.compile()
    return nc


def kernel(**inputs):
    x_news = np.asarray(inputs["x_news"], np.float32)
    ws = np.asarray(inputs["gat_n_ws"], np.float32)
    wd = np.asarray(inputs["gat_n_wd"], np.float32)
    a_s = np.asarray(inputs["gat_n_as"], np.float32)
    a_d = np.asarray(inputs["gat_n_ad"], np.float32)
    b = np.asarray(inputs["gat_n_b"], np.float32)
    w1 = np.asarray(inputs["lin1_w"], np.float32)
    b1 = np.asarray(inputs["lin1_b"], np.float32)
    w2 = np.asarray(inputs["lin2_w"], np.float32)
    b2 = np.asarray(inputs["lin2_b"], np.float32)
    n_id = np.asarray(inputs["n_id"], np.int64)
    news_indices = np.asarray(inputs["news_indices"], np.int64)

    in_maps, sched, shapes = _host_prep(
        x_news, ws, a_s, wd, a_d, b, w1, b1, w2, b2,
        inputs["links_src"], inputs["links_dst"])

    key = (shapes["S_HS"], shapes["S_ED"], shapes["C_TOT"],
           tuple(tuple(s["chunk_blocks"]) for s in sched))
    if key not in _CACHE:
        _CACHE.clear()
        _CACHE[key] = _build_program(sched, shapes)
    nc = _CACHE[key]

    from concourse.bass_utils import run_bass_kernel_spmd
    res = run_bass_kernel_spmd(nc, in_maps, core_ids=list(range(8)))

    out_full = np.empty((N_NEWS, 32), np.float32)
    for c in range(8):
        out_full[c * N_PER_CORE:(c + 1) * N_PER_CORE] = \
            res.results[c]["outt"][:, :N_PER_CORE].T

    local = np.searchsorted(n_id, news_indices)
    return out_full[local].astype(np.float32)


def _persistent_runner(nc, in_maps):
    """Build a reusable jitted 8-core executable with device-resident inputs.
    Returns (run_fn, fetch_fn) where run_fn() dispatches + blocks."""
    import jax
    import numpy as np_
    from jax.sharding import Mesh, PartitionSpec
    from jax.experimental.shard_map import shard_map
    import concourse.mybir as mybir
    from concourse.bass2jax import _bass_exec_p, install_neuronx_cc_hook

    install_neuronx_cc_hook()
    n_cores = len(in_maps)
    partition_name = nc.partition_id_tensor.name if nc.partition_id_tensor else None
    in_names, out_names, out_avals, zero_outs = [], [], [], []
    for alloc in nc.m.functions[0].allocations:
        if not isinstance(alloc, mybir.MemoryLocationSet):
            continue
        name = alloc.memorylocations[0].name
        if alloc.kind == "ExternalInput":
            if name != partition_name:
                in_names.append(name)
        elif alloc.kind == "ExternalOutput":
            shape = tuple(alloc.tensor_shape)
            dtype = mybir.dt.np(alloc.dtype)
            out_names.append(name)
            out_avals.append(jax.core.ShapedArray(shape, dtype))
            zero_outs.append(np_.zeros(shape, dtype))
    n_params = len(in_names)
    all_in = in_names + out_names
    if partition_name is not None:
        all_in.append(partition_name)

    def _body(*args):
        operands = list(args)
        if partition_name is not None:
            from concourse.bass2jax import partition_id_tensor
            operands.append(partition_id_tensor())
        return tuple(_bass_exec_p.bind(
            *operands, out_avals=tuple(out_avals), in_names=tuple(all_in),
            out_names=tuple(out_names), lowering_input_output_aliases=(),
            sim_require_finite=True, sim_require_nnan=True, nc=nc))

    devices = jax.devices()[:n_cores]
    mesh = Mesh(np_.asarray(devices), ("core",))
    nin = n_params + len(zero_outs)
    fn = jax.jit(shard_map(_body, mesh=mesh,
                           in_specs=(PartitionSpec("core"),) * nin,
                           out_specs=(PartitionSpec("core"),) * len(out_names),
                           check_rep=False))
    sh = jax.sharding.NamedSharding(mesh, PartitionSpec("core"))
    dev_in = [jax.device_put(
        np_.concatenate([np_.asarray(in_maps[c][n]) for c in range(n_cores)], axis=0), sh)
        for n in in_names]
    dev_zero = [jax.device_put(
        np_.zeros((n_cores * z.shape[0], *z.shape[1:]), z.dtype), sh) for z in zero_outs]

    state = {}

    def run_fn():
        out = fn(*dev_in, *dev_zero)
        jax.block_until_ready(out)
        state["out"] = out
        return out

    def fetch_fn():
        out = state["out"]
        return [{n: np_.asarray(out[i]).reshape(n_cores, *out_avals[i].shape)[c]
                 for i, n in enumerate(out_names)} for c in range(n_cores)]

    return run_fn, fetch_fn


def measure_hw_time(iters=12, **inputs):
    """Steady-state per-call wall time of the jitted executable, minus the
    dispatch baseline of a trivial program. Returns ns."""
    import time
    import concourse.bacc as bacc
    import concourse.mybir as mybir
    import concourse.tile as tile

    in_maps, sched, shapes = _host_prep(
        np.asarray(inputs["x_news"], np.float32),
        np.asarray(inputs["gat_n_ws"], np.float32), np.asarray(inputs["gat_n_as"], np.float32),
        np.asarray(inputs["gat_n_wd"], np.float32), np.asarray(inputs["gat_n_ad"], np.float32),
        np.asarray(inputs["gat_n_b"], np.float32),
        np.asarray(inputs["lin1_w"], np.float32), np.asarray(inputs["lin1_b"], np.float32),
        np.asarray(inputs["lin2_w"], np.float32), np.asarray(inputs["lin2_b"], np.float32),
        inputs["links_src"], inputs["links_dst"])
    key = (shapes["S_HS"], shapes["S_ED"], shapes["C_TOT"],
           tuple(tuple(s["chunk_blocks"]) for s in sched))
    if key not in _CACHE:
        _CACHE.clear()
        _CACHE[key] = _build_program(sched, shapes)
    nc = _CACHE[key]

    run_fn, _ = _persistent_runner(nc, in_maps)
    run_fn()  # compile + warm
    ts = []
    for _ in range(iters):
        t0 = time.perf_counter()
        run_fn()
        ts.append(time.perf_counter() - t0)
    t_kernel = min(ts)

    # trivial baseline program (same machinery, ~zero device work)
    f32 = mybir.dt.float32
    nb = bacc.Bacc("TRN2", target_bir_lowering=False, debug=False, num_devices=8)
    xi = nb.dram_tensor("xi", [128, 128], f32, kind="ExternalInput")
    xo = nb.dram_tensor("xo", [128, 128], f32, kind="ExternalOutput")
    with tile.TileContext(nb) as tc:
        with tc.tile_pool(name="p", bufs=1) as pool:
            t = pool.tile([128, 128], f32)
            nb.sync.dma_start(out=t[:], in_=xi.ap())
            nb.sync.dma_start(out=xo.ap(), in_=t[:])
    nb.compile()
    base_maps = [dict(xi=np.zeros((128, 128), np.float32))] * 8
    brun, _ = _persistent_runner(nb, base_maps)
    brun()
    bs = []
    for _ in range(iters):
        t0 = time.perf_counter()
        brun()
        bs.append(time.perf_counter() - t0)
    t_base = min(bs)
    print(f"  [timing] kernel call: {t_kernel*1e3:.2f} ms, baseline: {t_base*1e3:.2f} ms")
    return max(t_kernel - t_base, 0.0) * 1e9

